# revision 52
# baseline (speedup 1.0000x reference)
"""Trainium2 Bass kernel for nn_CrossAttentionBlock (LN -> MHA -> out-proj -> residual).

Sharding: 8 cores = 2 batches x 4 head-groups (2 heads each), no collectives.
Per core, streamed/pipelined design:
  - x DMA'd in 16 [128,512] pieces (token-quarter major); LN stats matmuls and
    squares overlap the DMA.
  - Projections run on RAW x (accumulated per token-quarter); the LN mean shift
    is folded in as a rank-1 matmul (u ox mu) and the per-token rsqrt scale is
    applied in one fused post-op (x rs_broadcast) that also casts to bf16/fp8.
    Q's beta-bias is folded via a second rank-1 (vq ox 1/rs); K's bias is
    dropped exactly (softmax row invariance); V's bias is applied on host.
  - Attention per (head, seq-half): bf16 scores, ACT exp -> fp8e4, AV matmul in
    fp8 DoubleRow mode (256-token contraction per pass). Sum-exp rides row 0 of
    the V stationary ([1|0pad|V] layout).
  - Normalize: DVE reciprocal of the sum-exp row, PE ones-broadcast matmul,
    DVE multiply. Out-proj per seq-half overlapped with the other half's
    attention; bf16 output DMA.
  - All ACT functions (Ln/Exp) are served by one activation table; redundant
    table loads are stripped post-compile.
Host sums the 4 partials per batch and adds bias + residual.
"""
import numpy as np

C = 512
SEQ = 2048
P = 128
NCH = 4          # c chunks of 128
NQ = 4           # token quarters of 512
QW = 512         # quarter width
DH = 64
HPC = 2          # heads per core
EPS = 1e-5
LNEXP_TABLE = 6  # natural_log_exp_and_others in act_info.json

_CACHE = {}
_LAST_IN_MAPS = None
FLAG_NO_SURGERY = False
FLAG_NO_DR = False
FLAG_NO_WARM = False


def _build():
    import concourse.bass as bass
    import concourse.tile as tile
    from concourse import bacc, mybir
    from concourse.masks import make_identity

    F32 = mybir.dt.float32
    F32R = mybir.dt.float32r
    BF16 = mybir.dt.bfloat16
    FP8 = mybir.dt.float8e4
    AF = mybir.ActivationFunctionType
    ALU = mybir.AluOpType
    DR = mybir.MatmulPerfMode.DoubleRow

    # Restrict the activation-table chooser to the one table that serves
    # every ACT function used here (Ln/Exp/Square) so a single table load is
    # emitted instead of per-function ping-pong reloads. Restored right after
    # compile.
    import concourse.bacc as bacc_mod
    _orig_gat = bacc_mod.get_activation_tables

    def _single_table(arch):
        tabs = list(_orig_gat(arch).items())
        return {name: (funcs if i == LNEXP_TABLE else set())
                for i, (name, funcs) in enumerate(tabs)}

    bacc_mod.get_activation_tables = _single_table
    try:
        nc = _build_inner(bacc, bass, tile, mybir, make_identity)
    finally:
        bacc_mod.get_activation_tables = _orig_gat
    return nc


def _build_inner(bacc, bass, tile, mybir, make_identity):
    F32 = mybir.dt.float32
    F32R = mybir.dt.float32r
    BF16 = mybir.dt.bfloat16
    FP8 = mybir.dt.float8e4
    AF = mybir.ActivationFunctionType
    ALU = mybir.AluOpType
    DR = mybir.MatmulPerfMode.DoubleRow

    nc = bacc.Bacc("TRN2", target_bir_lowering=False, debug=False,
                   enable_asserts=False, num_devices=8)

    xb_d = nc.dram_tensor("xb", [C, SEQ], F32, kind="ExternalInput").ap()
    aq_d = nc.dram_tensor("aq", [C, P], F32, kind="ExternalInput").ap()
    ak_d = nc.dram_tensor("ak", [C, P], F32, kind="ExternalInput").ap()
    av_d = nc.dram_tensor("av", [C, P], F32, kind="ExternalInput").ap()
    wo_d = nc.dram_tensor("wo", [P, C], F32, kind="ExternalInput").ap()
    u3_d = nc.dram_tensor("u3", [1, 3 * P], F32, kind="ExternalInput").ap()
    vq_d = nc.dram_tensor("vq", [P, 1], F32, kind="ExternalInput").ap()
    yp_d = nc.dram_tensor("yp", [C, SEQ], BF16, kind="ExternalOutput").ap()

    with tile.TileContext(nc) as tc:
        with tc.tile_pool(name="sb", bufs=1) as sb, \
             tc.tile_pool(name="rp", bufs=2) as rp, \
             tc.tile_pool(name="pa", bufs=2, space="PSUM") as pa:

            # ---- persistent SBUF state ----
            xt = [sb.tile([P, SEQ], F32R, tag=f"x{c}", name=f"xt{c}")
                  for c in range(NCH)]
            q_sb = sb.tile([P, SEQ], BF16, tag="qsb")
            k_sb = sb.tile([P, SEQ], BF16, tag="ksb")
            v_pre = sb.tile([P, SEQ], BF16, tag="vpre")
            attn = sb.tile([P, SEQ], BF16, tag="attn")
            mu_sb = sb.tile([1, NQ, QW], F32R, tag="mu")
            varr = sb.tile([1, NQ, QW], F32, tag="varr")
            lnv = sb.tile([1, NQ, QW], F32, tag="lnv")
            rs_sb = sb.tile([1, NQ, QW], F32R, tag="rs")
            rs_b = sb.tile([P, NQ, QW], F32R, tag="rsb")
            serec = sb.tile([1, 4, 1024], F32, tag="serec")

            def dma_x(q):
                qs = slice(q * QW, (q + 1) * QW)
                for c in range(NCH):
                    nc.sync.dma_start(
                        xt[c][:, qs],
                        xb_d[c * P:(c + 1) * P, qs].bitcast(F32R))

            # ---- x quarters 0/1 first, then early weights ----
            dma_x(0)
            dma_x(1)
            aw = {}
            for name, d in (("ak", ak_d), ("aq", aq_d), ("av", av_d)):
                t = sb.tile([P, NCH, P], F32R, tag=name, name=name)
                nc.sync.dma_start(
                    t[:], d.rearrange("(k p) m -> p k m", p=P).bitcast(F32R))
                aw[name] = t
            u3_sb = sb.tile([1, 3 * P], F32R, tag="u3")
            nc.sync.dma_start(u3_sb[:], u3_d[:, :].bitcast(F32R))
            uvec = {n: u3_sb[0:1, i * P:(i + 1) * P]
                    for i, n in enumerate(("uq", "uk", "uv"))}
            vq_c = sb.tile([P, 1], F32, tag="vqc")
            nc.sync.dma_start(vq_c[:], vq_d[:, :])

            # ---- constants (off SP queue); identity first so PE warmup can
            # start immediately, bulky v_sb memsets last ----
            ident_f = sb.tile([P, P], F32, tag="idf")
            make_identity(nc, ident_f[:])
            ident_r = sb.tile([P, P], F32R, tag="idr")
            nc.vector.tensor_copy(ident_r[:], ident_f[:])
            identb = sb.tile([P, P], BF16, tag="idb")
            nc.vector.tensor_copy(identb[:], ident_f[:])
            ones_mean = sb.tile([P, 1], F32, tag="om_f")
            nc.vector.memset(ones_mean[:], 1.0 / C)
            ones_mean_r = sb.tile([P, 1], F32R, tag="om")
            nc.vector.tensor_copy(ones_mean_r[:], ones_mean[:])
            ones_row = sb.tile([1, DH], F32, tag="or_f")
            nc.vector.memset(ones_row[:], 1.0)
            ones_row_r = sb.tile([1, DH], F32R, tag="orr")
            nc.vector.tensor_copy(ones_row_r[:], ones_row[:])
            eps_t = sb.tile([1, 1], F32, tag="eps")
            nc.vector.memset(eps_t[:], EPS)
            # V stationary: [j, jp, member, 2h x (1|pad|V)] fp8
            # (zero-filled in small pieces later so the big memset never
            # hogs the Pool engine ahead of make_identity)
            v_sb = sb.tile([P, 8, 2, 256], FP8, tag="vsb")

            def emit_vsb_init():
                for jp in range(8):
                    nc.vector.memset(v_sb[:, jp, :, :], 0.0)
                nc.gpsimd.memset(v_sb[:, :, :, 0:1], 1.0)
                nc.gpsimd.memset(v_sb[:, :, :, 128:129], 1.0)

            def emit_stats(q, sq_act, s1_tag, s2_tag, mu_act=None):
                qs = slice(q * QW, (q + 1) * QW)
                s1 = pa.tile([1, QW], F32, tag=s1_tag, name=f"s1q{q}",
                             bufs=1 if s1_tag == "av" else None)
                for c in range(NCH):
                    nc.tensor.matmul(s1[:], ones_mean_r[:], xt[c][:, qs],
                                     start=(c == 0), stop=(c == 3))
                s2 = pa.tile([1, QW], F32, tag=s2_tag, name=f"s2q{q}")
                for c in range(NCH):
                    sq = rp.tile([P, QW], F32R, tag="sq", name=f"sq{q}{c}",
                                  bufs=6)
                    if sq_act:
                        nc.scalar.activation(sq[:], xt[c][:, qs], AF.Square,
                                             bias=0.0, scale=1.0)
                    else:
                        nc.vector.tensor_tensor(sq[:], xt[c][:, qs],
                                                xt[c][:, qs], ALU.mult)
                    nc.tensor.matmul(s2[:], ones_mean_r[:], sq[:],
                                     start=(c == 0), stop=(c == 3))
                musq = rp.tile([1, QW], F32, tag="musq", name=f"musq{q}")
                if mu_act if mu_act is not None else sq_act:
                    nc.scalar.activation(mu_sb[0:1, q, :], s1[:], AF.Copy,
                                         bias=0.0, scale=1.0)
                    nc.scalar.activation(musq[:], mu_sb[0:1, q, :], AF.Square,
                                         bias=0.0, scale=1.0)
                else:
                    nc.vector.tensor_copy(mu_sb[0:1, q, :], s1[:])
                    nc.vector.tensor_tensor(musq[:], mu_sb[0:1, q, :],
                                            mu_sb[0:1, q, :], ALU.mult)
                nc.vector.tensor_tensor(varr[0:1, q, :], s2[:], musq[:],
                                        ALU.subtract)

            def emit_ln(q):
                nc.scalar.activation(lnv[0:1, q, :], varr[0:1, q, :], AF.Ln,
                                     bias=eps_t[0:1, :], scale=1.0)
                nc.scalar.activation(rs_sb[0:1, q, :], lnv[0:1, q, :], AF.Exp,
                                     bias=0.0, scale=-0.5)
                nc.gpsimd.partition_broadcast(rs_b[:, q, :], rs_sb[0:1, q, :],
                                              channels=P)

            def emit_proj1(name, q, tag):
                qs = slice(q * QW, (q + 1) * QW)
                out_sb = {"ak": k_sb, "aq": q_sb, "av": v_pre}[name]
                pp = pa.tile([P, QW], F32, tag=tag, name=f"pj{name}{q}",
                             bufs=1 if tag == "av" else None)
                for c in range(NCH):
                    nc.tensor.matmul(pp[:], aw[name][:, c, :],
                                     xt[c][:, qs], start=(c == 0),
                                     stop=False)
                u_t = uvec["u" + name[1]]
                nc.tensor.matmul(pp[:], u_t, mu_sb[0:1, q, :],
                                 start=False, stop=True)
                nc.vector.tensor_tensor(out_sb[:, qs], pp[:],
                                        rs_b[:, q, :], ALU.mult)
                if name == "aq":
                    # Q beta-bias (exact; zero when beta==0). K's drops in
                    # softmax row-normalization.
                    nc.vector.tensor_scalar(out=q_sb[:, qs], in0=q_sb[:, qs],
                                            scalar1=vq_c[:, 0:1], scalar2=None,
                                            op0=ALU.add)

            def emit_tr(q):
                for j in range(4):
                    jb = q * 4 + j
                    tr = pa.tile([P, P], BF16, tag="pj", name=f"tr{jb}")
                    nc.tensor.transpose(tr[:], v_pre[:, jb * P:(jb + 1) * P],
                                        identb[:])
                    nc.vector.tensor_copy(
                        v_sb[:, jb // 2, jb % 2, :]
                            .rearrange("p (h c) -> p h c", c=128)[:, :, 64:128],
                        tr[:].rearrange("p (h c) -> p h c", c=64))

            def emit_proj(q):
                for name in ("ak", "aq", "av"):
                    emit_proj1(name, q, "pj")
                emit_tr(q)

            av_tiles = {}

            def emit_att(h, ig, jp_lo, jp_hi):
                i0 = ig * 1024
                if (h, ig) not in av_tiles:
                    av_tiles[(h, ig)] = pa.tile([P, 1024], F32, tag="av",
                                                name=f"av{h}{ig}", bufs=1)
                av_ps = av_tiles[(h, ig)]
                hs = slice(h * DH, (h + 1) * DH)
                for jp in range(jp_lo, jp_hi):
                    e2 = rp.tile([P, 2, 1024], FP8, tag="e2",
                                 name=f"e2_{h}{ig}{jp}", bufs=4)
                    for m in range(2):
                        jb = jp * 2 + m
                        st = pa.tile([P, 1024], F32, tag="sc",
                                     name=f"st{h}{ig}{jb}")
                        for nb in range(2):
                            nc.tensor.matmul(
                                st[:, nb * 512:(nb + 1) * 512],
                                k_sb[hs, jb * P:(jb + 1) * P],
                                q_sb[hs, i0 + nb * 512:i0 + (nb + 1) * 512],
                                start=True, stop=True)
                        nc.scalar.activation(e2[:, m, :], st[:], AF.Exp,
                                             bias=0.0, scale=1.0)
                    for nb in range(2):
                        if FLAG_NO_DR:
                            for m2 in range(2):
                                nc.tensor.matmul(
                                    av_ps[:, nb * 512:(nb + 1) * 512],
                                    v_sb[:, jp, m2, h * 128:(h + 1) * 128],
                                    e2[:, m2, nb * 512:(nb + 1) * 512],
                                    start=(jp == 0 and m2 == 0),
                                    stop=(jp == 7 and m2 == 1))
                        else:
                            nc.tensor.matmul(
                                av_ps[:, nb * 512:(nb + 1) * 512],
                                v_sb[:, jp, :, h * 128:(h + 1) * 128],
                                e2[:, :, nb * 512:(nb + 1) * 512],
                                start=(jp == 0), stop=(jp == 7),
                                perf_mode=DR)

            def emit_norm(h, ig):
                i0 = ig * 1024
                av_ps = av_tiles.pop((h, ig))
                hig = 2 * ig + h
                rb = rp.tile([DH, 1024], F32, tag="rb", name=f"rb{h}{ig}")
                for nb in range(2):
                    s = slice(nb * 512, (nb + 1) * 512)
                    with nc.allow_low_precision(reason="sumexp recip row"):
                        nc.vector.reciprocal(serec[0:1, hig, s],
                                             av_ps[0:1, s])
                for nb in range(2):
                    s = slice(nb * 512, (nb + 1) * 512)
                    nc.gpsimd.partition_broadcast(rb[:, s],
                                                  serec[0:1, hig, s],
                                                  channels=DH)
                for nb in range(2):
                    s = slice(nb * 512, (nb + 1) * 512)
                    nc.vector.tensor_tensor(attn[h * DH:(h + 1) * DH,
                                                 i0 + nb * 512:
                                                 i0 + (nb + 1) * 512],
                                            av_ps[64:128, s], rb[:, s],
                                            ALU.mult)

            def emit_outproj(ig, tag="sc", final=False):
                i0 = ig * 1024
                dma_engines = [nc.sync, nc.scalar, nc.gpsimd, nc.sync]
                for m in range(4):
                    yp_sb = rp.tile([P, 1024], BF16, tag="yp",
                                    name=f"yp{ig}{m}", bufs=4)
                    if tag == "pj":
                        for nb in range(2):
                            yo = pa.tile([P, 512], F32, tag="pj",
                                         name=f"yo{ig}{m}{nb}")
                            nc.tensor.matmul(
                                yo[:],
                                wo_t[:, m * P:(m + 1) * P],
                                attn[:, i0 + nb * 512:i0 + (nb + 1) * 512],
                                start=True, stop=True)
                            nc.vector.tensor_copy(
                                yp_sb[:, nb * 512:(nb + 1) * 512], yo[:])
                    elif final:
                        for nb in range(2):
                            yo = pa.tile([P, 512], F32, tag="sc",
                                         name=f"yo{ig}{m}{nb}")
                            nc.tensor.matmul(
                                yo[:],
                                wo_t[:, m * P:(m + 1) * P],
                                attn[:, i0 + nb * 512:i0 + (nb + 1) * 512],
                                start=True, stop=True)
                            if (m + nb) % 2 == 0:
                                nc.vector.tensor_copy(
                                    yp_sb[:, nb * 512:(nb + 1) * 512], yo[:])
                            else:
                                nc.scalar.activation(
                                    yp_sb[:, nb * 512:(nb + 1) * 512], yo[:],
                                    AF.Copy, bias=0.0, scale=1.0)
                    else:
                        yo = pa.tile([P, 1024], F32, tag="sc",
                                     name=f"yo{ig}{m}")
                        for nb in range(2):
                            nc.tensor.matmul(
                                yo[:, nb * 512:(nb + 1) * 512],
                                wo_t[:, m * P:(m + 1) * P],
                                attn[:, i0 + nb * 512:i0 + (nb + 1) * 512],
                                start=True, stop=True)
                        nc.vector.tensor_copy(yp_sb[:], yo[:])
                    eng = dma_engines[m] if final else nc.sync
                    eng.dma_start(yp_d[m * P:(m + 1) * P, i0:i0 + 1024],
                                  yp_sb[:])

            # ---- emission schedule (per-engine queues run in order) ----
            # PE warmup: dummy matmuls ramp the tensor engine to full p-state
            # while x streams in (accumulated + consumed so they survive DCE).
            if not FLAG_NO_WARM:
                wu = pa.tile([P, P], F32, tag="av", name="wu", bufs=1)
                for w in range(14):
                    nc.tensor.matmul(wu[:], ident_r[:], ident_r[:],
                                     start=(w == 0), stop=(w == 13))
                wu_sink = sb.tile([1, 1], F32, tag="wus")
                nc.vector.tensor_copy(wu_sink[:], wu[0:1, 0:1])
            emit_stats(0, sq_act=True, s1_tag="sc", s2_tag="sc")
            emit_ln(0)
            emit_vsb_init()
            emit_stats(1, sq_act=False, s1_tag="av", s2_tag="pj",
                       mu_act=True)
            emit_ln(1)
            emit_proj1("ak", 0, "pj")
            emit_proj1("aq", 0, "sc")
            emit_proj1("ak", 1, "sc")
            emit_proj1("aq", 1, "av")
            emit_proj1("av", 0, "pj")
            emit_proj1("av", 1, "pj")
            dma_x(2)
            dma_x(3)
            emit_stats(2, sq_act=True, s1_tag="pj", s2_tag="pj",
                       mu_act=False)
            emit_tr(0)
            emit_tr(1)
            emit_att(0, 0, 0, 2)
            emit_ln(2)
            emit_stats(3, sq_act=False, s1_tag="pj", s2_tag="pj")
            emit_proj(2)
            emit_att(0, 0, 2, 4)
            emit_ln(3)
            wo_f = sb.tile([P, C], F32, tag="wof")
            nc.sync.dma_start(wo_f[:], wo_d[:, :])
            wo_t = sb.tile([P, C], BF16, tag="wo")
            nc.vector.tensor_copy(wo_t[:], wo_f[:])
            emit_proj(3)
            emit_att(0, 0, 4, 8)
            emit_att(1, 0, 0, 2)
            emit_norm(0, 0)
            emit_att(1, 0, 2, 8)
            emit_att(0, 1, 0, 2)
            emit_norm(1, 0)
            emit_att(0, 1, 2, 8)
            emit_outproj(0, tag="pj")
            emit_att(1, 1, 0, 2)
            emit_norm(0, 1)
            emit_att(1, 1, 2, 8)
            emit_norm(1, 1)
            emit_outproj(1, final=True)

    nc.compile()
    return nc


def kernel(x, Wq, Wk, Wv, Wo, bo, gamma, beta):
    from concourse import bass_utils

    x = np.asarray(x, np.float32)
    Wq, Wk, Wv, Wo = (np.asarray(w, np.float32) for w in (Wq, Wk, Wv, Wo))
    bo, gamma, beta = (np.asarray(v, np.float32) for v in (bo, gamma, beta))
    b = x.shape[0]
    xs = x.reshape(b, C, SEQ)

    s = DH ** -0.5
    aq_f = gamma[:, None] * Wq * s
    ak_f = gamma[:, None] * Wk
    av_f = gamma[:, None] * Wv
    vq_f = (Wq.T @ beta) * s
    vv_f = Wv.T @ beta

    if "nc" not in _CACHE:
        _CACHE["nc"] = _build()
    nc = _CACHE["nc"]

    in_maps = []
    for core in range(8):
        bi, hg = divmod(core, 4)
        cs = slice(hg * P, (hg + 1) * P)
        in_maps.append({
            "xb": np.ascontiguousarray(xs[bi]),
            "aq": np.ascontiguousarray(aq_f[:, cs]),
            "ak": np.ascontiguousarray(ak_f[:, cs]),
            "av": np.ascontiguousarray(av_f[:, cs]),
            "wo": np.ascontiguousarray(Wo[cs, :]),
            "u3": np.concatenate([-aq_f[:, cs].sum(0), -ak_f[:, cs].sum(0),
                                  -av_f[:, cs].sum(0)])[None, :]
                .astype(np.float32),
            "vq": vq_f[cs][:, None].astype(np.float32),
        })

    global _LAST_IN_MAPS
    _LAST_IN_MAPS = in_maps
    res = bass_utils.run_bass_kernel_spmd(nc, in_maps, core_ids=list(range(8)))
    bias_total = bo + Wo.T @ vv_f
    y = np.empty((b, C, SEQ), np.float32)
    for bi in range(b):
        acc = xs[bi] + bias_total[:, None]
        for hg in range(4):
            acc = acc + res.results[bi * 4 + hg]["yp"].astype(np.float32)
        y[bi] = acc
    return y.reshape(x.shape).astype(np.float32)
